# revision 1
# baseline (speedup 1.0000x reference)
"""Multi-level (FPN) DeformRoIPool (zero-offset == aligned RoIAlign) for Trainium2.

Strategy (8 NeuronCores, SPMD, one Bass program):
- Shard the 256 ROIs across cores (32 each); feature maps are preprocessed on
  host into per-ROI gather windows (channels-last pair-rows), so each core only
  uploads/reads the rows its ROIs touch.
- Per sample point (7x7 bins x 2x2 samples = 196 per ROI) one dma_gather
  element of 4KB covers the whole 2x2 bilinear patch: the window stores row
  pairs [F(y), F(y+1 clamped)] per (y, x) position (512 f32), and the gather
  element spans two consecutive x positions (1024 f32, overlapping stride).
- The weighted reduction over (sample, corner) -> (bin) runs on the PE as
  small matmuls with a host-built sparse weight matrix per ROI, accumulating
  in PSUM [49 bins, 256 c]. Host transposes [roi, bin, c] -> [roi, c, 7, 7].
"""
import os
import sys
import types

import numpy as np

OUT = 7
SR = 2
STRIDES = (4, 8, 16, 32)
FINEST = 56.0
IMG = 800.0
NLEV = 4
C = 256
N_ROIS = 256
N_CORES = 8
NROI_C = N_ROIS // N_CORES          # 32 rois per core
ROIS_PER_CALL = 4
NCALL = NROI_C // ROIS_PER_CALL     # gather calls per core
NSAMP = OUT * OUT * SR * SR         # 196 samples per roi
NREAL_CALL = ROIS_PER_CALL * NSAMP  # real gather idxs per call
NI_CALL = -(-NREAL_CALL // 16) * 16  # padded to x16 with trailing -1 (skipped)
NGRP_CALL = -(-NREAL_CALL // 128)   # slot groups per call
# flat (group, roi) matmul sets
GROUP_SETS = []
GROUP_K = []
for _g in range(NGRP_CALL):
    _lo, _hi = _g * 128, min((_g + 1) * 128, NREAL_CALL)
    GROUP_K.append(_hi - _lo)
    for _j in range(_lo // NSAMP, (_hi - 1) // NSAMP + 1):
        GROUP_SETS.append((_g, _j))
NSETS = len(GROUP_SETS)
WIN_R = 14 * 200                    # pair-row positions reserved per roi (l0 worst case)
WIN_STRIDE = WIN_R + 1              # +1 guard row per roi block
FEAT_SHAPES = [(2, 256, 200, 200), (2, 256, 100, 100), (2, 256, 50, 50), (2, 256, 25, 25)]


# ---------------------------------------------------------------------------
# BIR fix: this container's walrus rejects >1 embedded sem wait per
# instruction (2 on EventSemaphore). Split excess waits onto EventSemaphore
# carriers at serialization time.
# ---------------------------------------------------------------------------
def _install_bir_waitsplit():
    import orjson
    import concourse.bass as bass

    if getattr(bass.Bass, "_waitsplit_patched", False):
        return

    def _fix_blocks(blocks, counter):
        for blk in blocks:
            insts = blk.get("instructions")
            if insts:
                out = []
                for ins in insts:
                    si = ins.get("sync_info")
                    ow = (si or {}).get("on_wait") or []
                    limit = 2 if ins.get("opcode") == "EventSemaphore" else 1
                    if len(ow) > limit:
                        excess = ow[: len(ow) - limit]
                        si["on_wait"] = ow[len(ow) - limit:]
                        for i in range(0, len(excess), 2):
                            counter[0] += 1
                            out.append({
                                "name": f"I-waitsplit-{counter[0]}",
                                "opcode": "EventSemaphore",
                                "engine": ins["engine"],
                                "ins": [], "outs": [],
                                "debug": ins.get("debug", 0),
                                "sync_info": {"on_update": [], "on_wait": excess[i:i + 2]},
                            })
                    out.append(ins)
                blk["instructions"] = out
            if blk.get("blocks"):
                _fix_blocks(blk["blocks"], counter)

    orig = bass.Bass.to_json_bytes

    def to_json_bytes(self, *a, **kw):
        data = orig(self, *a, **kw)
        d = orjson.loads(data)
        counter = [0]
        for fn in d.get("functions", []):
            _fix_blocks(fn.get("blocks", []), counter)
        return orjson.dumps(d) if counter[0] else data

    bass.Bass.to_json_bytes = to_json_bytes
    bass.Bass._waitsplit_patched = True


# ---------------------------------------------------------------------------
# Host-side index / weight / window computation
# ---------------------------------------------------------------------------
def _roi_meta(rois):
    """Per-roi level + sample-grid floors and weights.

    Returns list of dicts with level l, batch b, and per-(i,si)/(j,sj) arrays.
    """
    scale_wh = np.sqrt((rois[:, 3] - rois[:, 1]) * (rois[:, 4] - rois[:, 2]))
    with np.errstate(divide="ignore"):
        tl = np.clip(np.floor(np.log2(scale_wh / FINEST + 1e-6)), 0, NLEV - 1)
    tl = (tl + 1e-5).astype(np.int32)
    g = np.arange(OUT, dtype=np.float64)[:, None] + (np.arange(SR, dtype=np.float64)[None, :] + 0.5) / SR
    metas = []
    for n in range(rois.shape[0]):
        l = int(tl[n])
        B, C_, H, W = FEAT_SHAPES[l]
        sc = 1.0 / STRIDES[l]
        x1 = rois[n, 1] * sc - 0.5
        y1 = rois[n, 2] * sc - 0.5
        rw = rois[n, 3] * sc - 0.5 - x1
        rh = rois[n, 4] * sc - 0.5 - y1
        y = y1 + (rh / OUT) * g  # [OUT, SR] sample y per (i, si)
        x = x1 + (rw / OUT) * g
        vy = (y > -1) & (y < H)
        vx = (x > -1) & (x < W)
        yc = np.clip(y, 0.0, H - 1)
        xc = np.clip(x, 0.0, W - 1)
        y0 = np.minimum(np.floor(yc).astype(np.int64), H - 1)
        x0 = np.minimum(np.floor(xc).astype(np.int64), W - 1)
        metas.append(dict(
            l=l, b=int(rois[n, 0]), H=H, W=W,
            y0=y0, x0=x0, ly=yc - y0, lx=xc - x0, vy=vy, vx=vx,
        ))
    return metas


def _build_core_inputs(feats_T, metas, core_rois):
    """Build win/idx/W tensors for one core's roi list (indices into metas)."""
    win = np.zeros((NROI_C * WIN_STRIDE + 1, 2 * C), np.float32)
    idx_all = np.full((NCALL, NI_CALL), -1, np.int16)
    wmat = np.zeros((NCALL, NSETS, 128, 4 * 49), np.float32)
    set_of = {(g_, j_): si_ for si_, (g_, j_) in enumerate(GROUP_SETS)}

    for rslot, n in enumerate(core_rois):
        m = metas[n]
        H, W = m["H"], m["W"]
        fT = feats_T[m["l"]][m["b"]]  # [H, W, C] channels-last view
        ys, yrank_of = np.unique(m["y0"]), {}
        for k, yv in enumerate(ys):
            yrank_of[yv] = k
        ysp1 = np.minimum(ys + 1, H - 1)
        nY = len(ys)
        # window block: rows [k*W + x] = [F(ys[k], x, :) | F(ys[k]+1c, x, :)]
        base = rslot * WIN_STRIDE
        blk = win[base:base + nY * W].reshape(nY, W, 2 * C)
        blk[:, :, :C] = fT[ys]
        blk[:, :, C:] = fT[ysp1]

        call, j = rslot // ROIS_PER_CALL, rslot % ROIS_PER_CALL
        jbase = j * WIN_STRIDE  # idx base within the call's 4-roi window span
        y0, x0, ly, lx = m["y0"], m["x0"], m["ly"], m["lx"]
        vy, vx = m["vy"], m["vx"]
        for i in range(OUT):
            for jj in range(OUT):
                for si in range(SR):
                    for sj in range(SR):
                        s = ((i * OUT + jj) * 4) + si * 2 + sj
                        slot = j * NSAMP + s
                        g_, p_ = slot // 128, slot % 128
                        yy0 = y0[i, si]
                        xx0 = x0[jj, sj]
                        idx_all[call, slot] = jbase + yrank_of[yy0] * W + xx0
                        v = (vy[i, si] and vx[jj, sj]) / (SR * SR)
                        hy = (1.0 - ly[i, si]) * v
                        lyv = ly[i, si] * v
                        hx = 1.0 - lx[jj, sj]
                        lxv = lx[jj, sj]
                        w0, w1, w2, w3 = hy * hx, lyv * hx, hy * lxv, lyv * lxv
                        if xx0 == W - 1:  # x1 clamps onto x0
                            w0, w2 = w0 + w2, 0.0
                            w1, w3 = w1 + w3, 0.0
                        b = s // 4
                        si_ = set_of[(g_, j)]
                        for q, w in enumerate((w0, w1, w2, w3)):
                            wmat[call, si_, p_, q * 49 + b] = w

    # idx layout per call: [128, NI/16], slot i -> [i%16, i//16], replicated x8
    idx_tiles = np.zeros((128, NCALL * (NI_CALL // 16)), np.int16)
    for c in range(NCALL):
        blk16 = idx_all[c].reshape(NI_CALL // 16, 16).T
        idx_tiles[:, c * (NI_CALL // 16):(c + 1) * (NI_CALL // 16)] = np.tile(blk16, (8, 1))
    return win, idx_tiles, wmat


def _build_core_inputs_fp16(feats_T, metas, core_rois):
    win, idx_tiles, wmat = _build_core_inputs(feats_T, metas, core_rois)
    return win.astype(np.float16), idx_tiles, wmat.astype(np.float16)


def _build_program():
    import concourse.bacc as bacc
    import concourse.mybir as mybir
    import concourse.tile as tile

    _install_bir_waitsplit()
    nc = bacc.Bacc("TRN2", debug=False, enable_asserts=True, num_devices=N_CORES)
    import concourse.bass as bass

    win_rows = NROI_C * WIN_STRIDE + 1
    win_d = nc.dram_tensor("win", [win_rows, 2 * C], mybir.dt.float16, kind="ExternalInput")
    idx_d = nc.dram_tensor("idx", [128, NCALL * (NI_CALL // 16)], mybir.dt.int16, kind="ExternalInput")
    w_d = nc.dram_tensor("wts", [NCALL * NSETS, 128, 4 * 49], mybir.dt.float16, kind="ExternalInput")
    out_d = nc.dram_tensor("out", [NROI_C, 49 * C], mybir.dt.float16, kind="ExternalOutput")


    with tile.TileContext(nc) as tc:
        with (
            tc.tile_pool(name="ip", bufs=1) as ip,
            tc.tile_pool(name="gp", bufs=8) as gp,
            tc.tile_pool(name="sp", bufs=3) as sp,
            tc.tile_pool(name="pp", bufs=8, space="PSUM") as pp,
        ):
            idx_t = ip.tile([128, NCALL * (NI_CALL // 16)], mybir.dt.int16)
            nc.sync.dma_start(idx_t[:], idx_d[:])
            wt = ip.tile([128, NCALL * NSETS * 4 * 49], mybir.dt.float16)
            nc.sync.dma_start(
                wt[:].rearrange("p (r w) -> p r w", w=4 * 49),
                w_d[:].rearrange("r p w -> p r w"),
            )
            for call in range(NCALL):
                g = gp.tile([128, NGRP_CALL * 4 * C], mybir.dt.float16, tag="g")
                # overlapping 4KB elems: row step 512 f32, elem 1024 f32
                src = bass.AP(
                    win_d[:].tensor,
                    call * ROIS_PER_CALL * WIN_STRIDE * (2 * C),
                    [[2 * C, ROIS_PER_CALL * WIN_STRIDE], [1, 4 * C]],
                )
                nc.gpsimd.dma_gather(
                    out_ap=g[:].rearrange("p (k c) -> p k c", c=4 * C),
                    in_ap=src,
                    idxs_ap=idx_t[:, call * (NI_CALL // 16):(call + 1) * (NI_CALL // 16)],
                    num_idxs=NI_CALL,
                    num_idxs_reg=NREAL_CALL,
                    elem_size=4 * C,
                    elem_step=2 * C,
                    single_packet=False,
                )
                st = sp.tile([49, ROIS_PER_CALL * C], mybir.dt.float16, tag="st")
                # first/last set index per roi j for start/stop flags
                firsts, lasts = {}, {}
                for si_, (g_, j_) in enumerate(GROUP_SETS):
                    firsts.setdefault(j_, si_)
                    lasts[j_] = si_
                ps_of = {j_: pp.tile([49, C], mybir.dt.float32, tag="ps", name=f"ps_{call}_{j_}") for j_ in range(ROIS_PER_CALL)}
                for si_, (g_, j_) in enumerate(GROUP_SETS):
                    K = GROUP_K[g_]
                    ps = ps_of[j_]
                    wb = (call * NSETS + si_) * 4 * 49
                    for q in range(4):
                        nc.tensor.matmul(
                            out=ps[:, :],
                            lhsT=wt[0:K, wb + q * 49:wb + (q + 1) * 49],
                            rhs=g[0:K, g_ * 4 * C + q * C:g_ * 4 * C + (q + 1) * C],
                            start=(si_ == firsts[j_] and q == 0),
                            stop=(si_ == lasts[j_] and q == 3),
                        )
                for j_ in range(ROIS_PER_CALL):
                    nc.vector.tensor_copy(st[:, j_ * C:(j_ + 1) * C], ps_of[j_][:])
                nc.sync.dma_start(
                    out_d[call * ROIS_PER_CALL:(call + 1) * ROIS_PER_CALL].rearrange(
                        "r (b c) -> b r c", c=C
                    ),
                    st[:].rearrange("b (r c) -> b r c", c=C),
                )
    nc.compile()
    return nc


def kernel(feat0, feat1, feat2, feat3, rois):
    from concourse.bass_utils import run_bass_kernel_spmd

    feats = [np.asarray(f, np.float32) for f in (feat0, feat1, feat2, feat3)]
    rois = np.asarray(rois, np.float32)
    # channels-last views per level/batch
    feats_T = [np.ascontiguousarray(f.transpose(0, 2, 3, 1)) for f in feats]
    metas = _roi_meta(rois)

    in_maps = []
    for core in range(N_CORES):
        core_rois = list(range(core * NROI_C, (core + 1) * NROI_C))
        win, idx_tiles, wmat = _build_core_inputs_fp16(feats_T, metas, core_rois)
        in_maps.append({"win": win, "idx": idx_tiles, "wts": wmat.reshape(NCALL * NSETS, 128, 4 * 49)})

    nc = _build_program()
    res = run_bass_kernel_spmd(nc, in_maps, core_ids=list(range(N_CORES)), trace=False)
    outs = []
    for core in range(N_CORES):
        o = res.results[core]["out"].astype(np.float32).reshape(NROI_C, 49, C)
        outs.append(np.ascontiguousarray(o.transpose(0, 2, 1)).reshape(NROI_C, C, OUT, OUT))
    return np.concatenate(outs, 0)


# Testing hook: emulate the device math in numpy (same win/idx/W data).
def emulate(feat0, feat1, feat2, feat3, rois):
    feats = [np.asarray(f, np.float32) for f in (feat0, feat1, feat2, feat3)]
    rois = np.asarray(rois, np.float32)
    feats_T = [np.ascontiguousarray(f.transpose(0, 2, 3, 1)) for f in feats]
    metas = _roi_meta(rois)
    out = np.zeros((N_ROIS, C, OUT, OUT), np.float32)
    for core in range(N_CORES):
        core_rois = list(range(core * NROI_C, (core + 1) * NROI_C))
        win, idx_tiles, wmat = _build_core_inputs(feats_T, metas, core_rois)
        winf = win.reshape(-1)
        for call in range(NCALL):
            idx_blk = idx_tiles[:16, call * (NI_CALL // 16):(call + 1) * (NI_CALL // 16)]
            slots = idx_blk.T.reshape(-1)
            base_off = call * ROIS_PER_CALL * WIN_STRIDE * (2 * C)
            G = np.zeros((NI_CALL, 4 * C), np.float32)
            for i in range(NREAL_CALL):
                st = base_off + int(slots[i]) * 2 * C
                G[i] = winf[st:st + 4 * C]
            accs = [np.zeros((49, C), np.float32) for _ in range(ROIS_PER_CALL)]
            for si_, (g_, j_) in enumerate(GROUP_SETS):
                K = GROUP_K[g_]
                W_ = wmat[call, si_]
                for q in range(4):
                    accs[j_] += W_[0:K, q * 49:(q + 1) * 49].T @ G[g_ * 128:g_ * 128 + K, q * C:(q + 1) * C]
            for j_ in range(ROIS_PER_CALL):
                r = core_rois[call * ROIS_PER_CALL + j_]
                out[r] = accs[j_].T.reshape(C, OUT, OUT)
    return out



# revision 9
# speedup vs baseline: 1.9840x; 1.9840x over previous
"""Multi-level (FPN) DeformRoIPool (zero-offset == aligned RoIAlign) for Trainium2.

Strategy (8 NeuronCores, SPMD, one Bass program):
- The bin/sample grid spacing is always < 2 px, so the set of pixels a ROI
  needs is exactly the dense bounding box of its sample corners. Host crops
  that box per ROI (channels-last fp16) and packs all of a core's crops into
  one contiguous "stream" [total_rows, 256] (row = one pixel, 512 B).
- Bilinear + sample-average reduction is separable: out[49, C] = W^T @ crop
  with W = Ay (x) Ax built per ROI on host. Device does K=128 matmuls
  (pixels on the partition dim) accumulating in PSUM [49, 256].
- ROIs are snake-dealt to cores by crop size; per-slot stream offsets are
  padded to the max across cores so the matmul schedule (group -> slot,
  start/stop) is identical on every core: SPMD-uniform program, raggedness
  lives in the data (stream contents + per-set weight tiles).
- Stream is stored pre-swizzled [128, G*256] so chunk DMAs are fully
  contiguous per partition (~1 MB each, near-peak HBM bandwidth). No gather.
"""
import numpy as np

OUT = 7
SR = 2
STRIDES = (4, 8, 16, 32)
FINEST = 56.0
NLEV = 4
C = 256
N_ROIS = 256
N_CORES = 8
NROI_C = N_ROIS // N_CORES          # 32 roi slots per core
CH_TARGET = 14                      # max groups per stream chunk DMA (~896 KB)
FEAT_SHAPES = [(2, 256, 200, 200), (2, 256, 100, 100), (2, 256, 50, 50), (2, 256, 25, 25)]


# ---------------------------------------------------------------------------
# BIR fix: this container's walrus rejects >1 embedded sem wait per
# instruction (2 on EventSemaphore). Split excess waits onto EventSemaphore
# carriers at serialization time.
# ---------------------------------------------------------------------------
def _install_bir_waitsplit():
    import orjson
    import concourse.bass as bass

    if getattr(bass.Bass, "_waitsplit_patched", False):
        return

    def _fix_blocks(blocks, counter):
        for blk in blocks:
            insts = blk.get("instructions")
            if insts:
                out = []
                for ins in insts:
                    si = ins.get("sync_info")
                    ow = (si or {}).get("on_wait") or []
                    limit = 2 if ins.get("opcode") == "EventSemaphore" else 1
                    if len(ow) > limit:
                        excess = ow[: len(ow) - limit]
                        si["on_wait"] = ow[len(ow) - limit:]
                        for i in range(0, len(excess), 2):
                            counter[0] += 1
                            out.append({
                                "name": f"I-waitsplit-{counter[0]}",
                                "opcode": "EventSemaphore",
                                "engine": ins["engine"],
                                "ins": [], "outs": [],
                                "debug": ins.get("debug", 0),
                                "sync_info": {"on_update": [], "on_wait": excess[i:i + 2]},
                            })
                    out.append(ins)
                blk["instructions"] = out
            if blk.get("blocks"):
                _fix_blocks(blk["blocks"], counter)

    orig = bass.Bass.to_json_bytes

    def to_json_bytes(self, *a, **kw):
        data = orig(self, *a, **kw)
        d = orjson.loads(data)
        counter = [0]
        for fn in d.get("functions", []):
            _fix_blocks(fn.get("blocks", []), counter)
        return orjson.dumps(d) if counter[0] else data

    bass.Bass.to_json_bytes = to_json_bytes
    bass.Bass._waitsplit_patched = True


# ---------------------------------------------------------------------------
# Host-side crop / weight computation
# ---------------------------------------------------------------------------
def _roi_meta(rois):
    """Per-roi level, crop bbox, and separable row/col weight matrices."""
    scale = np.sqrt((rois[:, 3] - rois[:, 1]) * (rois[:, 4] - rois[:, 2]))  # f32, as jax
    tl_f = np.clip(np.floor(np.log2(scale / np.float32(FINEST) + np.float32(1e-6))), 0, NLEV - 1)
    tl = (tl_f + 1e-5).astype(np.int32)
    g = np.arange(OUT, dtype=np.float64)[:, None] + (np.arange(SR, dtype=np.float64)[None, :] + 0.5) / SR
    metas = []
    for n in range(rois.shape[0]):
        l = int(tl[n])
        _, _, H, W = FEAT_SHAPES[l]
        sc = 1.0 / STRIDES[l]
        x1 = rois[n, 1] * sc - 0.5
        y1 = rois[n, 2] * sc - 0.5
        rw = rois[n, 3] * sc - 0.5 - x1
        rh = rois[n, 4] * sc - 0.5 - y1
        y = y1 + (rh / OUT) * g   # [OUT, SR]
        x = x1 + (rw / OUT) * g
        vy = (y > -1) & (y < H)
        vx = (x > -1) & (x < W)
        yc = np.clip(y, 0.0, H - 1)
        xc = np.clip(x, 0.0, W - 1)
        y0 = np.minimum(np.floor(yc).astype(np.int64), H - 1)
        x0 = np.minimum(np.floor(xc).astype(np.int64), W - 1)
        y1i = np.minimum(y0 + 1, H - 1)
        x1i = np.minimum(x0 + 1, W - 1)
        ly = yc - y0
        lx = xc - x0
        ymin, ymax = int(y0.min()), int(y1i.max())
        xmin, xmax = int(x0.min()), int(x1i.max())
        R, S = ymax - ymin + 1, xmax - xmin + 1
        Ay = np.zeros((R, OUT))
        Ax = np.zeros((S, OUT))
        for i in range(OUT):
            for si in range(SR):
                v = vy[i, si] * 0.5
                Ay[y0[i, si] - ymin, i] += (1.0 - ly[i, si]) * v
                Ay[y1i[i, si] - ymin, i] += ly[i, si] * v
                v = vx[i, si] * 0.5
                Ax[x0[i, si] - xmin, i] += (1.0 - lx[i, si]) * v
                Ax[x1i[i, si] - xmin, i] += lx[i, si] * v
        metas.append(dict(l=l, b=int(rois[n, 0]), ymin=ymin, xmin=xmin, R=R, S=S,
                          Ay=Ay, Ax=Ax, rows=R * S))
    return metas


def _plan(metas):
    """Snake-deal rois to cores by crop size; common per-slot row boundaries."""
    sizes = np.array([m["rows"] for m in metas])
    order = np.argsort(-sizes, kind="stable")
    cores = [[] for _ in range(N_CORES)]
    for k, n in enumerate(order):
        r, j = divmod(k, N_CORES)
        c = j if r % 2 == 0 else N_CORES - 1 - j
        cores[c].append(int(n))
    percore = np.array([[sizes[n] for n in cl] for cl in cores])       # [8, 32]
    bounds = np.cumsum(percore.max(axis=0)).astype(np.int64)           # common B_k
    total = int(bounds[-1])
    G = -(-total // 128)
    nch = -(-G // CH_TARGET)
    ch = -(-G // nch)
    G_pad = nch * ch
    # uniform set list: (slot, group, start, stop)
    sets = []
    for k in range(NROI_C):
        lo = 0 if k == 0 else int(bounds[k - 1])
        hi = int(bounds[k])
        g0, g1 = lo // 128, (hi - 1) // 128
        for gi in range(g0, g1 + 1):
            sets.append((k, gi, gi == g0, gi == g1))
    return cores, bounds, G, G_pad, ch, sets


def _build_core_inputs(feats_T, metas, core_rois, bounds, G_pad, sets):
    nsets = len(sets)
    stream = np.zeros((G_pad * 128, C), np.float16)
    wts = np.zeros((nsets, 128, 49), np.float16)
    set_idx = {}
    for s, (k, gi, _, _) in enumerate(sets):
        set_idx[(k, gi)] = s
    for k, n in enumerate(core_rois):
        m = metas[n]
        lo = 0 if k == 0 else int(bounds[k - 1])
        fT = feats_T[m["l"]][m["b"]]
        crop = fT[m["ymin"]:m["ymin"] + m["R"], m["xmin"]:m["xmin"] + m["S"], :]
        stream[lo:lo + m["rows"]] = crop.reshape(m["rows"], C)
        Wf = (m["Ay"][:, None, :, None] * m["Ax"][None, :, None, :]).reshape(m["rows"], 49)
        r = 0
        while r < m["rows"]:
            gr = lo + r
            gi = gr // 128
            p = gr - gi * 128
            take = min(128 - p, m["rows"] - r)
            wts[set_idx[(k, gi)], p:p + take] = Wf[r:r + take]
            r += take
    # pre-swizzle: stream row (g*128+p) -> [p, g*256 + c]
    stream_sw = np.ascontiguousarray(
        stream.reshape(G_pad, 128, C).transpose(1, 0, 2)).reshape(128, G_pad * C)
    wts_sw = np.ascontiguousarray(wts.transpose(1, 0, 2)).reshape(128, nsets * 49)
    return stream_sw, wts_sw


# ---------------------------------------------------------------------------
# Device program
# ---------------------------------------------------------------------------
def _build_program(G_pad, CH, sets):
    import concourse.bacc as bacc
    import concourse.mybir as mybir
    import concourse.tile as tile

    _install_bir_waitsplit()
    nc = bacc.Bacc("TRN2", debug=False, enable_asserts=True, num_devices=N_CORES)

    nsets = len(sets)
    nch = G_pad // CH
    # sets per chunk (uniform across cores)
    chunk_slo = []
    for c in range(nch):
        chunk_slo.append(sum(1 for (_, gi, _, _) in sets if gi < c * CH))
    chunk_slo.append(nsets)
    ns_max = max(chunk_slo[c + 1] - chunk_slo[c] for c in range(nch))

    stream_d = nc.dram_tensor("stream", [128, G_pad * C], mybir.dt.float16, kind="ExternalInput")
    wts_d = nc.dram_tensor("wts", [128, nsets * 49], mybir.dt.float16, kind="ExternalInput")
    out_d = nc.dram_tensor("out", [NROI_C, 49 * C], mybir.dt.float16, kind="ExternalOutput")

    with tile.TileContext(nc) as tc:
        with (
            tc.tile_pool(name="gp", bufs=3) as gp,
            tc.tile_pool(name="wp", bufs=3) as wp,
            tc.tile_pool(name="sp", bufs=2) as sp,
            tc.tile_pool(name="pp", bufs=8, space="PSUM") as pp,
        ):
            gt = {}
            wt = {}

            def emit_chunk(c):
                g = gp.tile([128, CH * C], mybir.dt.float16, tag="g", name=f"g_{c}")
                nc.sync.dma_start(g[:], stream_d[:, c * CH * C:(c + 1) * CH * C])
                w = wp.tile([128, ns_max * 49], mybir.dt.float16, tag="w", name=f"w_{c}")
                s0, s1 = chunk_slo[c], chunk_slo[c + 1]
                nc.sync.dma_start(w[:, 0:(s1 - s0) * 49], wts_d[:, s0 * 49:s1 * 49])
                gt[c] = g
                wt[c] = w

            emitted = -1
            ps = None
            st = None
            for s, (k, gi, first, last) in enumerate(sets):
                c = gi // CH
                while emitted < c:
                    emitted += 1
                    emit_chunk(emitted)
                if first:
                    ps = pp.tile([49, C], mybir.dt.float32, tag="ps", name=f"ps_{k}")
                if k % 8 == 0 and first:
                    st = sp.tile([49, 8 * C], mybir.dt.float16, tag="st", name=f"st_{k // 8}")
                nc.tensor.matmul(
                    out=ps[:, :],
                    lhsT=wt[c][:, (s - chunk_slo[c]) * 49:(s - chunk_slo[c] + 1) * 49],
                    rhs=gt[c][:, (gi - c * CH) * C:(gi - c * CH + 1) * C],
                    start=first,
                    stop=last,
                )
                if last:
                    nc.vector.tensor_copy(st[:, (k % 8) * C:(k % 8 + 1) * C], ps[:])
                    if k % 8 == 7:
                        nc.sync.dma_start(
                            out_d[k - 7:k + 1].rearrange("r (b c) -> b r c", c=C),
                            st[:].rearrange("b (r c) -> b r c", c=C),
                        )
    nc.compile()
    return nc


def kernel(feat0, feat1, feat2, feat3, rois):
    from concourse.bass_utils import run_bass_kernel_spmd

    feats = [np.asarray(f, np.float32) for f in (feat0, feat1, feat2, feat3)]
    rois = np.asarray(rois, np.float32)
    feats_T = [np.ascontiguousarray(f.transpose(0, 2, 3, 1)) for f in feats]
    metas = _roi_meta(rois)
    cores, bounds, G, G_pad, ch, sets = _plan(metas)

    in_maps = []
    for core in range(N_CORES):
        stream_sw, wts_sw = _build_core_inputs(feats_T, metas, cores[core], bounds, G_pad, sets)
        in_maps.append({"stream": stream_sw, "wts": wts_sw})

    nc = _build_program(G_pad, ch, sets)
    res = run_bass_kernel_spmd(nc, in_maps, core_ids=list(range(N_CORES)), trace=False)
    out = np.zeros((N_ROIS, C, OUT, OUT), np.float32)
    for core in range(N_CORES):
        o = res.results[core]["out"].astype(np.float32).reshape(NROI_C, 49, C)
        o = o.transpose(0, 2, 1).reshape(NROI_C, C, OUT, OUT)
        for k, n in enumerate(cores[core]):
            out[n] = o[k]
    return out


# Testing hook: emulate the device math in numpy (same stream/weight data).
def emulate(feat0, feat1, feat2, feat3, rois):
    feats = [np.asarray(f, np.float32) for f in (feat0, feat1, feat2, feat3)]
    rois = np.asarray(rois, np.float32)
    feats_T = [np.ascontiguousarray(f.transpose(0, 2, 3, 1)) for f in feats]
    metas = _roi_meta(rois)
    cores, bounds, G, G_pad, ch, sets = _plan(metas)
    out = np.zeros((N_ROIS, C, OUT, OUT), np.float32)
    for core in range(N_CORES):
        stream_sw, wts_sw = _build_core_inputs(feats_T, metas, cores[core], bounds, G_pad, sets)
        stream = stream_sw.reshape(128, G_pad, C).transpose(1, 0, 2).astype(np.float32)
        wts = wts_sw.reshape(128, len(sets), 49).transpose(1, 0, 2).astype(np.float32)
        accs = {}
        for s, (k, gi, first, last) in enumerate(sets):
            if first:
                accs[k] = np.zeros((49, C), np.float32)
            accs[k] += wts[s].T @ stream[gi]
            if last:
                n = cores[core][k]
                out[n] = accs[k].T.reshape(C, OUT, OUT)
    return out


# revision 12
# speedup vs baseline: 2.0948x; 1.0559x over previous
"""Multi-level (FPN) DeformRoIPool (zero-offset == aligned RoIAlign) for Trainium2.

Strategy (8 NeuronCores, SPMD, one Bass program):
- The bin/sample grid spacing is always < 2 px, so the set of pixels a ROI
  needs is exactly the dense bounding box of its sample corners. Host crops
  that box per ROI (channels-last fp16) and packs all of a core's crops into
  one contiguous "stream" [total_rows, 256] (row = one pixel, 512 B).
- Bilinear + sample-average reduction is separable: out[49, C] = W^T @ crop
  with W = Ay (x) Ax built per ROI on host. Device does K=128 matmuls
  (pixels on the partition dim) accumulating in PSUM [49, 256].
- ROIs are snake-dealt to cores by crop size; per-slot stream offsets are
  padded to the max across cores so the matmul schedule (group -> slot,
  start/stop) is identical on every core: SPMD-uniform program, raggedness
  lives in the data (stream contents + per-set weight tiles).
- Stream is stored pre-swizzled [128, G*256] so chunk DMAs are fully
  contiguous per partition (~1 MB each, near-peak HBM bandwidth). No gather.
"""
import numpy as np

OUT = 7
SR = 2
STRIDES = (4, 8, 16, 32)
FINEST = 56.0
NLEV = 4
C = 256
N_ROIS = 256
N_CORES = 8
NROI_C = N_ROIS // N_CORES          # 32 roi slots per core
CH_TARGET = 14                      # max groups per stream chunk DMA (~896 KB)
FEAT_SHAPES = [(2, 256, 200, 200), (2, 256, 100, 100), (2, 256, 50, 50), (2, 256, 25, 25)]


# ---------------------------------------------------------------------------
# BIR fix: this container's walrus rejects >1 embedded sem wait per
# instruction (2 on EventSemaphore). Split excess waits onto EventSemaphore
# carriers at serialization time.
# ---------------------------------------------------------------------------
def _install_bir_waitsplit():
    import orjson
    import concourse.bass as bass

    if getattr(bass.Bass, "_waitsplit_patched", False):
        return

    def _fix_blocks(blocks, counter):
        for blk in blocks:
            insts = blk.get("instructions")
            if insts:
                out = []
                for ins in insts:
                    si = ins.get("sync_info")
                    ow = (si or {}).get("on_wait") or []
                    limit = 2 if ins.get("opcode") == "EventSemaphore" else 1
                    if len(ow) > limit:
                        excess = ow[: len(ow) - limit]
                        si["on_wait"] = ow[len(ow) - limit:]
                        for i in range(0, len(excess), 2):
                            counter[0] += 1
                            out.append({
                                "name": f"I-waitsplit-{counter[0]}",
                                "opcode": "EventSemaphore",
                                "engine": ins["engine"],
                                "ins": [], "outs": [],
                                "debug": ins.get("debug", 0),
                                "sync_info": {"on_update": [], "on_wait": excess[i:i + 2]},
                            })
                    out.append(ins)
                blk["instructions"] = out
            if blk.get("blocks"):
                _fix_blocks(blk["blocks"], counter)

    orig = bass.Bass.to_json_bytes

    def to_json_bytes(self, *a, **kw):
        data = orig(self, *a, **kw)
        d = orjson.loads(data)
        counter = [0]
        for fn in d.get("functions", []):
            _fix_blocks(fn.get("blocks", []), counter)
        return orjson.dumps(d) if counter[0] else data

    bass.Bass.to_json_bytes = to_json_bytes
    bass.Bass._waitsplit_patched = True


# ---------------------------------------------------------------------------
# Host-side crop / weight computation
# ---------------------------------------------------------------------------
def _roi_meta(rois):
    """Per-roi level, crop bbox, and separable row/col weight matrices."""
    scale = np.sqrt((rois[:, 3] - rois[:, 1]) * (rois[:, 4] - rois[:, 2]))  # f32, as jax
    tl_f = np.clip(np.floor(np.log2(scale / np.float32(FINEST) + np.float32(1e-6))), 0, NLEV - 1)
    tl = (tl_f + 1e-5).astype(np.int32)
    g = np.arange(OUT, dtype=np.float64)[:, None] + (np.arange(SR, dtype=np.float64)[None, :] + 0.5) / SR
    metas = []
    for n in range(rois.shape[0]):
        l = int(tl[n])
        _, _, H, W = FEAT_SHAPES[l]
        sc = 1.0 / STRIDES[l]
        x1 = rois[n, 1] * sc - 0.5
        y1 = rois[n, 2] * sc - 0.5
        rw = rois[n, 3] * sc - 0.5 - x1
        rh = rois[n, 4] * sc - 0.5 - y1
        y = y1 + (rh / OUT) * g   # [OUT, SR]
        x = x1 + (rw / OUT) * g
        vy = (y > -1) & (y < H)
        vx = (x > -1) & (x < W)
        yc = np.clip(y, 0.0, H - 1)
        xc = np.clip(x, 0.0, W - 1)
        y0 = np.minimum(np.floor(yc).astype(np.int64), H - 1)
        x0 = np.minimum(np.floor(xc).astype(np.int64), W - 1)
        y1i = np.minimum(y0 + 1, H - 1)
        x1i = np.minimum(x0 + 1, W - 1)
        ly = yc - y0
        lx = xc - x0
        ymin, ymax = int(y0.min()), int(y1i.max())
        xmin, xmax = int(x0.min()), int(x1i.max())
        R, S = ymax - ymin + 1, xmax - xmin + 1
        Ay = np.zeros((R, OUT))
        Ax = np.zeros((S, OUT))
        for i in range(OUT):
            for si in range(SR):
                v = vy[i, si] * 0.5
                Ay[y0[i, si] - ymin, i] += (1.0 - ly[i, si]) * v
                Ay[y1i[i, si] - ymin, i] += ly[i, si] * v
                v = vx[i, si] * 0.5
                Ax[x0[i, si] - xmin, i] += (1.0 - lx[i, si]) * v
                Ax[x1i[i, si] - xmin, i] += lx[i, si] * v
        metas.append(dict(l=l, b=int(rois[n, 0]), ymin=ymin, xmin=xmin, R=R, S=S,
                          Ay=Ay, Ax=Ax, rows=R * S))
    return metas


def _plan(metas):
    """Snake-deal rois to cores by crop size; common per-slot row boundaries."""
    sizes = np.array([m["rows"] for m in metas])
    order = np.argsort(-sizes, kind="stable")
    cores = [[] for _ in range(N_CORES)]
    for k, n in enumerate(order):
        r, j = divmod(k, N_CORES)
        c = j if r % 2 == 0 else N_CORES - 1 - j
        cores[c].append(int(n))
    percore = np.array([[sizes[n] for n in cl] for cl in cores])       # [8, 32]
    bounds = np.cumsum(percore.max(axis=0)).astype(np.int64)           # common B_k
    total = int(bounds[-1])
    G = -(-total // 128)
    nch = -(-G // CH_TARGET)
    ch = -(-G // nch)
    G_pad = nch * ch
    # uniform set list: (slot, group, start, stop)
    sets = []
    for k in range(NROI_C):
        lo = 0 if k == 0 else int(bounds[k - 1])
        hi = int(bounds[k])
        g0, g1 = lo // 128, (hi - 1) // 128
        for gi in range(g0, g1 + 1):
            sets.append((k, gi, gi == g0, gi == g1))
    return cores, bounds, G, G_pad, ch, sets


def _build_core_inputs(feats_T, metas, core_rois, bounds, G_pad, sets):
    nsets = len(sets)
    stream = np.zeros((G_pad * 128, C), np.float16)
    wts = np.zeros((nsets, 128, 49), np.float16)
    set_idx = {}
    for s, (k, gi, _, _) in enumerate(sets):
        set_idx[(k, gi)] = s
    for k, n in enumerate(core_rois):
        m = metas[n]
        lo = 0 if k == 0 else int(bounds[k - 1])
        fT = feats_T[m["l"]][m["b"]]
        crop = fT[m["ymin"]:m["ymin"] + m["R"], m["xmin"]:m["xmin"] + m["S"], :]
        stream[lo:lo + m["rows"]] = crop.reshape(m["rows"], C)
        Wf = (m["Ay"][:, None, :, None] * m["Ax"][None, :, None, :]).reshape(m["rows"], 49)
        r = 0
        while r < m["rows"]:
            gr = lo + r
            gi = gr // 128
            p = gr - gi * 128
            take = min(128 - p, m["rows"] - r)
            wts[set_idx[(k, gi)], p:p + take] = Wf[r:r + take]
            r += take
    # pre-swizzle: stream row (g*128+p) -> [p, g*256 + c]
    stream_sw = np.ascontiguousarray(
        stream.reshape(G_pad, 128, C).transpose(1, 0, 2)).reshape(128, G_pad * C)
    wts_sw = np.ascontiguousarray(wts.transpose(1, 0, 2)).reshape(128, nsets * 49)
    return stream_sw, wts_sw


# ---------------------------------------------------------------------------
# Device program
# ---------------------------------------------------------------------------
def _build_program(G_pad, CH, sets):
    import concourse.bacc as bacc
    import concourse.mybir as mybir
    import concourse.tile as tile

    _install_bir_waitsplit()
    nc = bacc.Bacc("TRN2", debug=False, enable_asserts=True, num_devices=N_CORES)

    nsets = len(sets)
    nch = G_pad // CH
    # sets per chunk (uniform across cores)
    chunk_slo = []
    for c in range(nch):
        chunk_slo.append(sum(1 for (_, gi, _, _) in sets if gi < c * CH))
    chunk_slo.append(nsets)
    ns_max = max(chunk_slo[c + 1] - chunk_slo[c] for c in range(nch))

    stream_d = nc.dram_tensor("stream", [128, G_pad * C], mybir.dt.float16, kind="ExternalInput")
    wts_d = nc.dram_tensor("wts", [128, nsets * 49], mybir.dt.float16, kind="ExternalInput")
    out_d = nc.dram_tensor("out", [NROI_C, 49 * C], mybir.dt.float16, kind="ExternalOutput")

    with tile.TileContext(nc) as tc:
        with (
            tc.tile_pool(name="gp", bufs=4) as gp,
            tc.tile_pool(name="wp", bufs=4) as wp,
            tc.tile_pool(name="sp", bufs=2) as sp,
            tc.tile_pool(name="pp", bufs=8, space="PSUM") as pp,
        ):
            gt = {}
            wt = {}

            def emit_chunk(c):
                # alternate the two HWDGE rings (SP via nc.sync, ACT via
                # nc.scalar) so stream chunks overlap instead of FIFO-serializing
                eng_a = nc.sync if c % 2 == 0 else nc.scalar
                eng_b = nc.scalar if c % 2 == 0 else nc.sync
                g = gp.tile([128, CH * C], mybir.dt.float16, tag="g", name=f"g_{c}")
                eng_a.dma_start(g[:], stream_d[:, c * CH * C:(c + 1) * CH * C])
                w = wp.tile([128, ns_max * 49], mybir.dt.float16, tag="w", name=f"w_{c}")
                s0, s1 = chunk_slo[c], chunk_slo[c + 1]
                eng_b.dma_start(w[:, 0:(s1 - s0) * 49], wts_d[:, s0 * 49:s1 * 49])
                gt[c] = g
                wt[c] = w

            emitted = -1
            ps = None
            st = None
            for s, (k, gi, first, last) in enumerate(sets):
                c = gi // CH
                while emitted < c:
                    emitted += 1
                    emit_chunk(emitted)
                if first:
                    ps = pp.tile([49, C], mybir.dt.float32, tag="ps", name=f"ps_{k}")
                if k % 8 == 0 and first:
                    st = sp.tile([49, 8 * C], mybir.dt.float16, tag="st", name=f"st_{k // 8}")
                nc.tensor.matmul(
                    out=ps[:, :],
                    lhsT=wt[c][:, (s - chunk_slo[c]) * 49:(s - chunk_slo[c] + 1) * 49],
                    rhs=gt[c][:, (gi - c * CH) * C:(gi - c * CH + 1) * C],
                    start=first,
                    stop=last,
                )
                if last:
                    nc.vector.tensor_copy(st[:, (k % 8) * C:(k % 8 + 1) * C], ps[:])
                    if k % 8 == 7:
                        eng = nc.sync if (k // 8) % 2 == 0 else nc.scalar
                        eng.dma_start(
                            out_d[k - 7:k + 1].rearrange("r (b c) -> b r c", c=C),
                            st[:].rearrange("b (r c) -> b r c", c=C),
                        )
    nc.compile()
    return nc


def kernel(feat0, feat1, feat2, feat3, rois):
    from concourse.bass_utils import run_bass_kernel_spmd

    feats = [np.asarray(f, np.float32) for f in (feat0, feat1, feat2, feat3)]
    rois = np.asarray(rois, np.float32)
    feats_T = [np.ascontiguousarray(f.transpose(0, 2, 3, 1)) for f in feats]
    metas = _roi_meta(rois)
    cores, bounds, G, G_pad, ch, sets = _plan(metas)

    in_maps = []
    for core in range(N_CORES):
        stream_sw, wts_sw = _build_core_inputs(feats_T, metas, cores[core], bounds, G_pad, sets)
        in_maps.append({"stream": stream_sw, "wts": wts_sw})

    nc = _build_program(G_pad, ch, sets)
    res = run_bass_kernel_spmd(nc, in_maps, core_ids=list(range(N_CORES)), trace=False)
    out = np.zeros((N_ROIS, C, OUT, OUT), np.float32)
    for core in range(N_CORES):
        o = res.results[core]["out"].astype(np.float32).reshape(NROI_C, 49, C)
        o = o.transpose(0, 2, 1).reshape(NROI_C, C, OUT, OUT)
        for k, n in enumerate(cores[core]):
            out[n] = o[k]
    return out


# Testing hook: emulate the device math in numpy (same stream/weight data).
def emulate(feat0, feat1, feat2, feat3, rois):
    feats = [np.asarray(f, np.float32) for f in (feat0, feat1, feat2, feat3)]
    rois = np.asarray(rois, np.float32)
    feats_T = [np.ascontiguousarray(f.transpose(0, 2, 3, 1)) for f in feats]
    metas = _roi_meta(rois)
    cores, bounds, G, G_pad, ch, sets = _plan(metas)
    out = np.zeros((N_ROIS, C, OUT, OUT), np.float32)
    for core in range(N_CORES):
        stream_sw, wts_sw = _build_core_inputs(feats_T, metas, cores[core], bounds, G_pad, sets)
        stream = stream_sw.reshape(128, G_pad, C).transpose(1, 0, 2).astype(np.float32)
        wts = wts_sw.reshape(128, len(sets), 49).transpose(1, 0, 2).astype(np.float32)
        accs = {}
        for s, (k, gi, first, last) in enumerate(sets):
            if first:
                accs[k] = np.zeros((49, C), np.float32)
            accs[k] += wts[s].T @ stream[gi]
            if last:
                n = cores[core][k]
                out[n] = accs[k].T.reshape(C, OUT, OUT)
    return out
